# revision 26
# baseline (speedup 1.0000x reference)
"""Trainium2 Bass kernel for nn_Attention_57380763075267 (sparse_attention).

Reference computation (B=4, N=2048, DIM=512, H=8, HD=64):
    qkv = (x @ qkv_w.T) -> q, v   (k == q)
    attn = softmax(mask(q @ q.T * HD**-0.5))
    out  = (attn @ v)  -> reshape -> @ proj_w.T + proj_b
Sharding: 8 cores = (batch b in 0..3) x (query-half ih in 0..1).
Each core computes the full attention for its 1024 query rows of batch b
(all 8 heads on-core). No collectives; host slices/reassembles.

Engine-balance design (v3):
  * Scores / P tiles live in the TRANSPOSED domain [j(keys) x i(queries)];
    every matmul has a 512-wide moving operand; score matmuls for a head
    pair run concurrently on the PE via row tiling (K=64 at rows 0-63 /
    64-127).
  * Per (head-pair, jt): ONE [128,1024] exp on ACT (both heads fused) and
    ONE [128,1024] mask multiply on DVE (mask duplicated host-side).
  * Softmax without row-max (scores*SCALE is O(3)); Z via ones-column of V.
  * Z normalization: DVE reciprocal of the two Z rows -> [1,1024] SBUF,
    gpsimd partition_broadcast across partitions, two DVE muls. (No K=1
    matmul broadcast, no extra copies.)
  * q_sb / v_sb double-buffered across For_i reps so the next rep's
    projection bootstrap overlaps this rep's attention tail; mask DMA split
    by ic half for the same reason.
  * Projection PSUM tiles share the po pool; the qf pool is dedicated to
    the q/v projection chunks.
"""

import numpy as np

import concourse.bacc as bacc
import concourse.tile as tile
from concourse import mybir
from concourse.bass_utils import run_bass_kernel_spmd

B, N, DIM, H = 4, 2048, 512, 8
HD = DIM // H          # 64
SCALE = HD ** -0.5     # 0.125
I = N // 2             # 1024 queries per core
NCORES = 8

F32 = mybir.dt.float32
F16 = mybir.dt.float16
EXP = mybir.ActivationFunctionType.Exp

QDT = F16


def build_nc(reps=1):
    """Build the per-core program. reps>1 wraps the body in a HW loop
    (used only for wall-clock benchmarking by repetition)."""
    import contextlib

    nc = bacc.Bacc(None)

    xTp = nc.declare_dram_parameter("xTp", [DIM, N], QDT, isOutput=False).ap()
    wT = nc.declare_dram_parameter("wT", [DIM, 2 * DIM], QDT, isOutput=False).ap()
    pwT = nc.declare_dram_parameter("pwT", [DIM, DIM], F16, isOutput=False).ap()
    pb = nc.declare_dram_parameter("pb", [DIM], F32, isOutput=False).ap()
    # keep mask^T, transposed+permuted and duplicated: row j holds
    # [k(ic0)|k(ic0)|k(ic1)|k(ic1)], each 512 wide
    keepTp = nc.declare_dram_parameter("keepTp", [N, 2 * I], F16, isOutput=False).ap()
    outT = nc.declare_dram_parameter("outT", [DIM, I], F32, isOutput=True).ap()

    with tile.TileContext(nc) as tc:
        with (
            tc.tile_pool(name="singles", bufs=1) as singles,
            tc.tile_pool(name="qbuf", bufs=2) as qbuf,
            tc.tile_pool(name="vbuf", bufs=2) as vbuf,
            tc.tile_pool(name="pt", bufs=8) as pt_pool,
            tc.tile_pool(name="small", bufs=2) as small,
            tc.tile_pool(name="rzb", bufs=2) as rzb_pool,
            tc.tile_pool(name="fin", bufs=2) as fin_pool,
            tc.tile_pool(name="ps", bufs=2, space="PSUM") as ps_pool,
            tc.tile_pool(name="po", bufs=3, space="PSUM") as po_pool,
            tc.tile_pool(name="qf", bufs=1, space="PSUM") as qf_pool,
            tc.For_i(0, reps, 1) if reps > 1 else contextlib.nullcontext(),
        ):
            # ---- resident SBUF tensors ----
            w_sb = singles.tile([128, 4, 2 * DIM], QDT)     # qkv_w.T  (c-chunk major)
            x_sb = singles.tile([128, 4, N], QDT)           # x[b].T   (c-chunk major)
            keep_sb = singles.tile([128, 16, 2 * I], F16)   # dup keep mask^T (j-tile major)
            q_sb = qbuf.tile([128, 4, N], QDT)              # q^T (head-pair major)
            v_sb = vbuf.tile([128, 16, H * (HD + 1)], F16)  # V': 8 x [64 v | 1] per j
            oh_sb = singles.tile([128, 4, I], F16)          # normalized O^T (hd-chunks)
            pw_sb = singles.tile([128, 4, DIM], F16)        # proj_w.T (hd-chunk major)
            pb_sb = singles.tile([128, 4], F32)             # proj bias (d-tile major)
            warm = singles.tile([1, 8], F32)
            warm_in = singles.tile([1, 8], F32)

            # ---- phase-1-critical input DMAs ----
            xv = xTp.rearrange("(t p) n -> p t n", p=128)
            wv = wT.rearrange("(t p) o -> p t o", p=128)
            kv = keepTp.rearrange("(t p) i -> p t i", p=128)
            nc.sync.dma_start(out=x_sb[:, :, 0:512], in_=xv[:, :, 0:512])
            nc.sync.dma_start(out=w_sb[:, :, 0:DIM], in_=wv[:, :, 0:DIM])
            nc.sync.dma_start(out=w_sb[:, :, DIM : 2 * DIM], in_=wv[:, :, DIM : 2 * DIM])
            nc.sync.dma_start(out=x_sb[:, :, 512:1024], in_=xv[:, :, 512:1024])
            nc.sync.dma_start(out=x_sb[:, :, 1024:1536], in_=xv[:, :, 1024:1536])
            nc.sync.dma_start(out=x_sb[:, :, 1536:2048], in_=xv[:, :, 1536:2048])
            # mask tiles ride the SWDGE queue, split by ic half so the next
            # rep's ic0 reload can start as soon as this rep's ic0 is done
            for kq in range(2):
                nc.gpsimd.dma_start(
                    out=keep_sb[:, 0:8, kq * I : (kq + 1) * I],
                    in_=kv[:, 0:8, kq * I : (kq + 1) * I],
                )
                nc.gpsimd.dma_start(
                    out=keep_sb[:, 8:16, kq * I : (kq + 1) * I],
                    in_=kv[:, 8:16, kq * I : (kq + 1) * I],
                )

            # warm the ACT exp table while DMAs run
            nc.gpsimd.memset(warm_in, 1.0)
            nc.scalar.activation(out=warm, in_=warm_in, func=EXP, scale=1.0)

            ones_sb = singles.tile([1, HD], F16)
            nc.gpsimd.memset(ones_sb, 1.0)

            # ones columns of V' (column 64 of every 65-wide head group)
            vview = v_sb.rearrange("p t (h e) -> p t h e", e=HD + 1)
            nc.gpsimd.memset(vview[:, :, :, HD : HD + 1], 1.0)

            # ---- q/v projection chunks ----
            def q_chunk2(ot, nch2):
                """Wide q double-chunk through the (pre-attention idle) ps
                pool: 8 matmuls, one [128,1024] copy."""
                psq = ps_pool.tile([128, 1024], F32, tag="ps")
                for h2 in range(2):
                    for c4 in range(4):
                        nc.tensor.matmul(
                            psq[:, h2 * 512 : h2 * 512 + 512],
                            lhsT=w_sb[:, c4, ot * 128 : (ot + 1) * 128],
                            rhs=x_sb[:, c4, (2 * nch2 + h2) * 512 : (2 * nch2 + h2 + 1) * 512],
                            start=(c4 == 0),
                            stop=(c4 == 3),
                        )
                nc.vector.tensor_copy(
                    q_sb[:, ot, nch2 * 1024 : (nch2 + 1) * 1024], psq
                )

            def q_chunk(ot, nch, pool):
                psq = pool.tile([128, 512], F32, tag="qf" if pool is qf_pool else "po")
                for c4 in range(4):
                    nc.tensor.matmul(
                        psq,
                        lhsT=w_sb[:, c4, ot * 128 : (ot + 1) * 128],
                        rhs=x_sb[:, c4, nch * 512 : (nch + 1) * 512],
                        start=(c4 == 0),
                        stop=(c4 == 3),
                    )
                nc.vector.tensor_copy(q_sb[:, ot, nch * 512 : (nch + 1) * 512], psq)

            def v_chunk(nt, pool):
                psv = pool.tile([128, 512], F32, tag="qf" if pool is qf_pool else "po")
                for c4 in range(4):
                    nc.tensor.matmul(
                        psv,
                        lhsT=x_sb[:, c4, nt * 128 : (nt + 1) * 128],
                        rhs=w_sb[:, c4, DIM : 2 * DIM],
                        start=(c4 == 0),
                        stop=(c4 == 3),
                    )
                nc.vector.tensor_copy(
                    vview[:, nt, :, 0:HD],
                    psv.rearrange("p (h e) -> p h e", e=HD),
                )

            # pre-phase: q row-block 0 (needed by the first scores) via the
            # still-idle ps pool, double-buffered
            q_chunk2(0, 0)
            q_chunk2(0, 1)

            # ---- phase 2 + 3: attention, then projection per query chunk ----
            for ic in range(2):
                for hp in range(4):
                    if ic == 0 and hp == 1:
                        nc.sync.dma_start(
                            out=pw_sb, in_=pwT.rearrange("(t p) d -> p t d", p=128)
                        )
                        nc.sync.dma_start(
                            out=pb_sb, in_=pb.rearrange("(t p) -> p t", p=128)
                        )
                    po_e = po_pool.tile([128, 512], F32, tag="po")
                    po_o = po_pool.tile([128, 512], F32, tag="po")

                    def attn(pts, jt):
                        # O^T accumulation (row 64 collects Z via ones column)
                        nc.tensor.matmul(
                            po_e[0 : HD + 1, :],
                            lhsT=v_sb[:, jt, (2 * hp) * 65 : (2 * hp) * 65 + 65],
                            rhs=pts[0],
                            start=(jt == 0),
                            stop=(jt == 15),
                        )
                        nc.tensor.matmul(
                            po_o[0 : HD + 1, :],
                            lhsT=v_sb[:, jt, (2 * hp + 1) * 65 : (2 * hp + 1) * 65 + 65],
                            rhs=pts[1],
                            start=(jt == 0),
                            stop=(jt == 15),
                        )

                    pending = None  # attn deferred one tile: PE never stalls
                    for jt in range(16):
                        # q row-block hp+1 is produced one block early (skewed)
                        # so no block ever waits on its own q tiles
                        if ic == 0 and hp < 3 and jt % 4 == 3:
                            q_chunk(hp + 1, jt // 4, qf_pool)
                        if ic == 0 and hp == 0:
                            # v tiles alternate between the qf pool and the
                            # idle 3rd po slot for 2-deep pipelining
                            v_chunk(jt, po_pool if jt % 2 else qf_pool)
                        ps2 = ps_pool.tile([128, 1024], F32, tag="ps")
                        for g in range(2):
                            nc.tensor.matmul(
                                ps2[:, 512 * g : 512 * g + 512],
                                lhsT=q_sb[64 * g : 64 * g + 64, hp,
                                          jt * 128 : (jt + 1) * 128],
                                rhs=q_sb[64 * g : 64 * g + 64, hp,
                                         ic * 512 : (ic + 1) * 512],
                                start=True,
                                stop=True,
                            )
                        ptw = pt_pool.tile([128, 1024], F16, tag="pt")
                        # fused pair: one exp, one mask multiply
                        nc.scalar.activation(
                            out=ptw, in_=ps2, func=EXP, scale=float(SCALE)
                        )
                        nc.vector.tensor_mul(
                            ptw, ptw, keep_sb[:, jt, ic * 1024 : ic * 1024 + 1024]
                        )
                        pts = [ptw[:, 0:512], ptw[:, 512:1024]]
                        if pending is not None:
                            attn(*pending)
                        pending = (pts, jt)
                    attn(*pending)
                    # normalization: oh = O^T * (1/Z); 1/Z broadcast across
                    # partitions on GPSIMD. Per-head chains so po_e frees
                    # as early as possible (shortens the po recycle path).
                    # 1/Z rows -> f16 -> K=1 matmul broadcasts both heads'
                    # reciprocals across partitions in one PSUM tile
                    rz = small.tile([1, 1024], F32, tag="rz")
                    nc.vector.reciprocal(rz[0:1, 0:512], po_e[HD : HD + 1, :])
                    nc.vector.reciprocal(rz[0:1, 512:1024], po_o[HD : HD + 1, :])
                    rzh = small.tile([1, 1024], F16, tag="rzh")
                    nc.vector.tensor_copy(rzh, rz)
                    przp = qf_pool.tile([128, 512], F32, tag="qf")
                    nc.tensor.matmul(
                        przp[0:64, :],
                        lhsT=ones_sb[0:1, :],
                        rhs=rzh[0:1, 0:512],
                        start=True,
                        stop=True,
                    )
                    nc.tensor.matmul(
                        przp[64:128, :],
                        lhsT=ones_sb[0:1, :],
                        rhs=rzh[0:1, 512:1024],
                        start=True,
                        stop=True,
                    )
                    rzr = rzb_pool.tile([128, 512], F32, tag="rzb")
                    nc.vector.tensor_copy(rzr, przp)
                    nc.vector.tensor_mul(
                        oh_sb[0:64, hp, ic * 512 : (ic + 1) * 512],
                        po_e[0:HD, :],
                        rzr[0:64, :],
                    )
                    nc.vector.tensor_mul(
                        oh_sb[64:128, hp, ic * 512 : (ic + 1) * 512],
                        po_o[0:HD, :],
                        rzr[64:128, :],
                    )

                # projection for this query chunk (overlaps next chunk's
                # attention; PSUM tiles rotate through the qf pool)
                for dt4 in range(4):
                    pf = qf_pool.tile([128, 512], F32, tag="qf")
                    for hp in range(4):
                        nc.tensor.matmul(
                            pf,
                            lhsT=pw_sb[:, hp, dt4 * 128 : (dt4 + 1) * 128],
                            rhs=oh_sb[:, hp, ic * 512 : (ic + 1) * 512],
                            start=(hp == 0),
                            stop=(hp == 3),
                        )
                    fin = fin_pool.tile([128, 512], F32, tag="fin")
                    nc.vector.tensor_scalar_add(fin, pf, pb_sb[:, dt4 : dt4 + 1])
                    nc.sync.dma_start(
                        out=outT[dt4 * 128 : (dt4 + 1) * 128, ic * 512 : (ic + 1) * 512],
                        in_=fin,
                    )

    nc.compile()
    return nc


def prep_inputs(x, qkv_w, proj_w, proj_b, freq_attn_mask):
    """Build the 8 per-core input maps (host-side slicing/permutation)."""
    x = np.asarray(x, dtype=np.float32)
    qkv_w = np.asarray(qkv_w, dtype=np.float32)
    proj_w = np.asarray(proj_w, dtype=np.float32)
    proj_b = np.asarray(proj_b, dtype=np.float32)
    mask = np.asarray(freq_attn_mask)

    qdt = np.float16
    wT = np.ascontiguousarray(qkv_w.T).astype(qdt)           # [512, 1024]
    pwT = np.ascontiguousarray(proj_w.T).astype(np.float16)  # [512, 512] f16
    keepT = np.ascontiguousarray((1 - mask).T.astype(np.float16))  # [2048 j, 2048 i]

    in_maps = []
    for c in range(NCORES):
        b, ih = c // 2, c % 2
        lo, hi = ih * I, (ih + 1) * I
        perm = np.r_[lo:hi, 0:lo, hi:N]  # queries first, rest after
        xT = x[b].T  # [512, 2048]
        kc = keepT[perm][:, lo:hi]                      # [N, I]
        # duplicate each 512-wide ic chunk: [k0|k0|k1|k1]
        kc2 = np.broadcast_to(
            kc.reshape(N, 2, 1, 512), (N, 2, 2, 512)
        ).reshape(N, 2 * I)
        in_maps.append(
            {
                "xTp": np.ascontiguousarray(xT[:, perm]).astype(qdt),
                "wT": wT,
                "pwT": pwT,
                "pb": proj_b,
                "keepTp": np.ascontiguousarray(kc2),
            }
        )
    return in_maps


def assemble(results):
    out = np.empty((B, N, DIM), dtype=np.float32)
    for c in range(NCORES):
        b, ih = c // 2, c % 2
        out[b, ih * I : (ih + 1) * I, :] = results[c]["outT"].T
    return out


_NC_CACHE = None


def kernel(x, qkv_w, proj_w, proj_b, freq_attn_mask):
    global _NC_CACHE
    if _NC_CACHE is None:
        _NC_CACHE = build_nc()
    nc = _NC_CACHE
    in_maps = prep_inputs(x, qkv_w, proj_w, proj_b, freq_attn_mask)
    res = run_bass_kernel_spmd(nc, in_maps, list(range(NCORES)))
    return assemble(res.results)
